# revision 1
# baseline (speedup 1.0000x reference)
"""Trainium2 Bass kernel for nn_CustomMLPLayer_13408887898971 (topk_masking).

Computes (matching reference.py):
    scores = sum_s relu(x[0,s,:])          # [d_ff]
    idx    = top_k(scores, K)              # K = 4403
    out    = x[..., idx] @ W[:, idx].T     # [1, S, d_model]

Key identity: gathering the same top-K columns of x and W and contracting
equals a dense contraction with the non-top-K columns masked to zero:
    out = (x * m) @ W.T  with  m[j] = scores[j] >= (K-th largest score)

Strategy (8 NeuronCores, tensor-parallel over d_model):
  - host: transpose x and W to j-major (contraction on partitions),
    shard W.T by d_model columns (512 per core), x.T replicated.
  - device, per core:
      phase A: partial scores over this core's 256-token shard
               (ACT relu with accum_out), fp32
      phase B: AllReduce partial scores across the 8 cores (44KB)
      phase C: exact K-th largest via radix-16 binary search on the f32
               bit pattern (non-negative floats order like ints); then
               mask = scores >= threshold
      phase D: masked dense GEMM: psum[d,s] += (W.T[jt] * mask[jt]).T
               @ x.T[jt,s] accumulated over 86 j-tiles
  - host: concat per-core [512, 2048] out.T shards, transpose.
"""

import numpy as np

N_CORES = 8

FULL_CFG = dict(
    dff=11008,
    s=2048,
    d=4096,
    k=4403,
    name="full",
)

# matmul operand dtype: "f32" (exact, 4 cyc/row) or "bf16" (1 cyc/row)
MM_DTYPE = "f32"

_cache = {}


def _build_program(cfg):
    """Build + compile the 8-core SPMD bass program. Returns (nc, meta)."""
    from concourse import bacc, tile
    import concourse.bass as bass
    import concourse.mybir as mybir
    import concourse.bass_isa as bass_isa

    dt = mybir.dt
    Alu = mybir.AluOpType

    DFF = cfg["dff"]
    S = cfg["s"]
    D = cfg["d"]
    K = cfg["k"]
    DSH = D // N_CORES           # d_model cols per core
    SSH = S // N_CORES           # score-token shard per core
    JT = DFF // 128              # j tiles
    SCH = min(512, S)            # moving free dim per matmul
    NSCH = S // SCH              # s chunks
    DT = max(1, DSH // 128)      # d tiles per core (lhsT free dim 128)
    assert DSH % 128 == 0 or DSH < 128
    DW = min(128, DSH)           # width of a d tile

    mmdt = dt.float32 if cfg.get("mm_dtype", MM_DTYPE) == "f32" else dt.bfloat16

    nc = bacc.Bacc(
        "TRN2", target_bir_lowering=False, debug=False, num_devices=N_CORES
    )

    # I/O (per-core tensors; in_maps provide per-core data)
    xs = nc.dram_tensor("xs", [DFF, SSH], dt.float32, kind="ExternalInput").ap()
    xt = nc.dram_tensor("xt", [DFF, S], mmdt, kind="ExternalInput").ap()
    wt = nc.dram_tensor("wt", [DFF, DSH], mmdt, kind="ExternalInput").ap()
    outT = nc.dram_tensor("outT", [DSH, S], dt.float32, kind="ExternalOutput").ap()
    debug = cfg.get("debug", False)
    if debug:
        dbg_scores = nc.dram_tensor(
            "dbg_scores", [128, DFF // 128], dt.float32, kind="ExternalOutput"
        ).ap()
        dbg_mask = nc.dram_tensor(
            "dbg_mask", [128, DFF // 128], dt.float32, kind="ExternalOutput"
        ).ap()
        dbg_thr = nc.dram_tensor(
            "dbg_thr", [128, 1], dt.int32, kind="ExternalOutput"
        ).ap()
        dbg_partial = nc.dram_tensor(
            "dbg_partial", [128, 2 * (DFF // 128)], dt.float32, kind="ExternalOutput"
        ).ap()
        dbg_cnts = nc.dram_tensor(
            "dbg_cnts", [128, 15], dt.float32, kind="ExternalOutput"
        ).ap()
        dbg_cntr = nc.dram_tensor(
            "dbg_cntr", [128, 15], dt.float32, kind="ExternalOutput"
        ).ap()
        dbg_sel = nc.dram_tensor(
            "dbg_sel", [128, 15], dt.float32, kind="ExternalOutput"
        ).ap()
        dbg_digf = nc.dram_tensor(
            "dbg_digf", [128, 1], dt.float32, kind="ExternalOutput"
        ).ap()
        dbg_digi = nc.dram_tensor(
            "dbg_digi", [128, 1], dt.int32, kind="ExternalOutput"
        ).ap()
        dbg_thrs = nc.dram_tensor(
            "dbg_thrs", [128, 8], dt.int32, kind="ExternalOutput"
        ).ap()
        dbg_cand = nc.dram_tensor(
            "dbg_cand", [128, 15], dt.int32, kind="ExternalOutput"
        ).ap()
        dbg_ge = nc.dram_tensor(
            "dbg_ge", [128, DFF // 128], dt.float32, kind="ExternalOutput"
        ).ap()
        dbg_cnts_all = nc.dram_tensor(
            "dbg_cnts_all", [128, 8 * 15], dt.float32, kind="ExternalOutput"
        ).ap()
        dbg_cntr_all = nc.dram_tensor(
            "dbg_cntr_all", [128, 8 * 15], dt.float32, kind="ExternalOutput"
        ).ap()

    with tile.TileContext(nc) as tc:
        with (
            tc.tile_pool(name="persist", bufs=1) as pp,
            tc.tile_pool(name="xs_p", bufs=3) as xsp,
            tc.tile_pool(name="relu_p", bufs=3) as rlp,
            tc.tile_pool(name="xt_p", bufs=4) as xtp,
            tc.tile_pool(name="wt_p", bufs=4) as wtp,
            tc.tile_pool(name="wm_p", bufs=4) as wmp,
            tc.tile_pool(name="out_p", bufs=3) as otp,
            tc.tile_pool(name="psum", bufs=2, space="PSUM") as psp,
            tc.tile_pool(name="dram", bufs=1, space="DRAM") as drp,
        ):
            # ---- persistent small tiles ----
            # partial holds [hsum | rsum]: integer-part sums (exact in f32)
            # and fractional-residue sums of relu(x)*1024 per j.
            partial = pp.tile([128, 2 * JT], dt.float32, tag="partial")
            scores = pp.tile([128, JT], dt.float32, tag="scores")
            mask = pp.tile([128, JT], dt.float32, tag="mask")
            thr = pp.tile([128, 1], dt.int32, tag="thr")
            cand = pp.tile([128, 1], dt.int32, tag="cand")
            ge_scr = pp.tile([128, JT], dt.float32, tag="ge_scr")
            cnts = pp.tile([128, 15], dt.float32, tag="cnts")
            cntr = pp.tile([128, 15], dt.float32, tag="cntr")
            sel = pp.tile([128, 15], dt.float32, tag="sel")
            digf = pp.tile([128, 1], dt.float32, tag="digf")
            digi = pp.tile([128, 1], dt.int32, tag="digi")

            # ---- phase A: partial scores over this core's token shard ----
            # Scores must effectively match fp64 accuracy: the reference's
            # top-K boundary gap (~4e-4 abs) is only a few f32 ULP, so a
            # plain f32 running sum (noise ~3e-4) flips boundary neurons.
            # Two-limb trick on r = relu(x)*1024: the integer part h sums
            # EXACTLY in f32 (all partials are integers < 2^24), and the
            # fractional part r1 < 1 sums with noise ~1e-6 relative.
            for t in range(JT):
                st = xsp.tile([128, SSH], dt.float32)
                nc.sync.dma_start(st[:], xs[t * 128 : (t + 1) * 128, :])
                rt = rlp.tile([128, SSH], dt.float32, tag="rt")
                nc.scalar.activation(
                    rt[:],
                    st[:],
                    mybir.ActivationFunctionType.Relu,
                    scale=1024.0,
                )
                # h = round-to-nearest-int(r) via the +2^23 trick (r < 2^13),
                # exact in f32; |r1| = |r - h| <= 0.5.
                tmpt = rlp.tile([128, SSH], dt.float32, tag="tmpt")
                nc.vector.tensor_scalar(
                    out=tmpt[:],
                    in0=rt[:],
                    scalar1=float(2.0**23),
                    scalar2=None,
                    op0=Alu.add,
                )
                ht = rlp.tile([128, SSH], dt.float32, tag="ht")
                nc.vector.tensor_scalar(
                    out=ht[:],
                    in0=tmpt[:],
                    scalar1=float(2.0**23),
                    scalar2=0.0,
                    op0=Alu.subtract,
                    op1=Alu.add,
                    accum_out=partial[:, t : t + 1],
                )
                r1t = rlp.tile([128, SSH], dt.float32, tag="r1t")
                if cfg.get("use_ttr", False):
                    nc.vector.tensor_tensor_reduce(
                        out=r1t[:],
                        in0=rt[:],
                        in1=ht[:],
                        scale=1.0,
                        scalar=0.0,
                        op0=Alu.subtract,
                        op1=Alu.add,
                        accum_out=partial[:, JT + t : JT + t + 1],
                    )
                else:
                    nc.vector.tensor_tensor(
                        out=r1t[:], in0=rt[:], in1=ht[:], op=Alu.subtract
                    )
                    nc.vector.tensor_reduce(
                        out=partial[:, JT + t : JT + t + 1],
                        in_=r1t[:],
                        axis=mybir.AxisListType.X,
                        op=Alu.add,
                    )

            # ---- phase B: AllReduce partial sums across cores ----
            cc_in = drp.tile([128, 2 * JT], dt.float32)
            cc_out = drp.tile([128, 2 * JT], dt.float32)
            nc.sync.dma_start(cc_in[:], partial[:])
            nc.gpsimd.collective_compute(
                "AllReduce",
                Alu.add,
                replica_groups=[list(range(N_CORES))],
                ins=[cc_in.opt()],
                outs=[cc_out.opt()],
            )
            hr = pp.tile([128, 2 * JT], dt.float32, tag="hr")
            nc.sync.dma_start(hr[:], cc_out[:])
            # scores = (hsum + rsum) * 2^-10  (single final rounding)
            nc.vector.tensor_tensor(
                out=scores[:], in0=hr[:, :JT], in1=hr[:, JT:], op=Alu.add
            )
            nc.vector.tensor_scalar(
                out=scores[:],
                in0=scores[:],
                scalar1=float(2.0**-10),
                scalar2=None,
                op0=Alu.mult,
            )

            # ---- phase C: K-th largest via radix-16 search on f32 bits ----
            # scores >= 0, so f32 bit patterns order like int32. Candidates are
            # built in int32 bit space, and compared in f32 space (bitcast the
            # candidate): order is identical for non-negative values, and
            # candidates that land in the inf/nan range compare as "no score
            # >= cand", matching the int compare.
            # NOTE: the DVE ALU evaluates int32 tensor ops in f32 arithmetic,
            # so bit-space increments below ULP(thr_bits ~ 2^30) = 128 are
            # rounded away. The int-bit stage therefore only resolves bits
            # 7..30 (increments are multiples of 128 -> exact in f32); the
            # low 7 bits are resolved in float space using exact ULP steps.
            nc.vector.memset(thr[:], 0)

            def count_round(make_cand, ncand, upd):
                """One radix round: count candidates, pick digit, update thr."""
                for r in range(1, ncand + 1):
                    make_cand(r)
                    nc.vector.tensor_scalar(
                        out=ge_scr[:],
                        in0=scores[:],
                        scalar1=candf[:],
                        scalar2=0.0,
                        op0=Alu.is_ge,
                        op1=Alu.add,
                        accum_out=cnts[:, r - 1 : r],
                    )
                nc.gpsimd.partition_all_reduce(
                    cntr[:, :ncand],
                    cnts[:, :ncand],
                    channels=128,
                    reduce_op=bass_isa.ReduceOp.add,
                )
                nc.vector.tensor_scalar(
                    out=sel[:, :ncand],
                    in0=cntr[:, :ncand],
                    scalar1=float(K),
                    scalar2=None,
                    op0=Alu.is_ge,
                )
                nc.vector.tensor_reduce(
                    out=digf[:],
                    in_=sel[:, :ncand],
                    axis=mybir.AxisListType.X,
                    op=Alu.add,
                )
                upd()

            candf = pp.tile([128, 1], dt.float32, tag="candf")
            thr_f = pp.tile([128, 1], dt.float32, tag="thr_f")
            ulp = pp.tile([128, 1], dt.float32, tag="ulp")
            step = pp.tile([128, 1], dt.float32, tag="step")

            # --- int-bit stage: bits 7..30, radix 16 ---
            for shift in (27, 23, 19, 15, 11, 7):

                def make_cand_int(r, shift=shift):
                    nc.vector.tensor_scalar(
                        out=cand[:],
                        in0=thr[:],
                        scalar1=r << shift,
                        scalar2=None,
                        op0=Alu.add,
                    )
                    # view the int candidate as f32 for the compare
                    nc.vector.tensor_scalar(
                        out=candf[:],
                        in0=cand[:].bitcast(dt.float32),
                        scalar1=0.0,
                        scalar2=None,
                        op0=Alu.add,
                    )

                def upd_int(shift=shift):
                    nc.vector.tensor_scalar(
                        out=digi[:],
                        in0=digf[:],
                        scalar1=float(1 << shift),
                        scalar2=None,
                        op0=Alu.mult,
                    )
                    nc.vector.tensor_tensor(
                        out=thr[:], in0=thr[:], in1=digi[:], op=Alu.add
                    )

                count_round(make_cand_int, 15, upd_int)

            # --- float stage: low 7 bits with exact ULP steps ---
            # ulp = (bitcast(thr+128) - bitcast(thr)) / 128 (exact powers of 2)
            nc.vector.tensor_scalar(
                out=cand[:], in0=thr[:], scalar1=128, scalar2=None, op0=Alu.add
            )
            nc.vector.tensor_tensor(
                out=ulp[:],
                in0=cand[:].bitcast(dt.float32),
                in1=thr[:].bitcast(dt.float32),
                op=Alu.subtract,
            )
            nc.vector.tensor_scalar(
                out=ulp[:],
                in0=ulp[:],
                scalar1=1.0 / 128.0,
                scalar2=None,
                op0=Alu.mult,
            )
            nc.vector.tensor_scalar(
                out=thr_f[:],
                in0=thr[:].bitcast(dt.float32),
                scalar1=0.0,
                scalar2=None,
                op0=Alu.add,
            )

            for mult_, ncand in ((16, 7), (1, 15)):

                def make_cand_f(r, mult_=mult_):
                    nc.vector.tensor_scalar(
                        out=step[:],
                        in0=ulp[:],
                        scalar1=float(r * mult_),
                        scalar2=None,
                        op0=Alu.mult,
                    )
                    nc.vector.tensor_tensor(
                        out=candf[:], in0=thr_f[:], in1=step[:], op=Alu.add
                    )

                def upd_f(mult_=mult_):
                    nc.vector.tensor_scalar(
                        out=digf[:],
                        in0=digf[:],
                        scalar1=float(mult_),
                        scalar2=None,
                        op0=Alu.mult,
                    )
                    nc.vector.tensor_tensor(
                        out=step[:], in0=digf[:], in1=ulp[:], op=Alu.mult
                    )
                    nc.vector.tensor_tensor(
                        out=thr_f[:], in0=thr_f[:], in1=step[:], op=Alu.add
                    )

                count_round(make_cand_f, ncand, upd_f)

            # mask[j] = scores >= thr_f  (0.0/1.0 f32)
            nc.vector.tensor_scalar(
                out=mask[:],
                in0=scores[:],
                scalar1=thr_f[:],
                scalar2=None,
                op0=Alu.is_ge,
            )

            if debug:
                nc.sync.dma_start(dbg_scores[:], scores[:])
                nc.sync.dma_start(dbg_mask[:], mask[:])
                nc.sync.dma_start(dbg_thr[:], thr_f[:].bitcast(dt.int32))
                nc.sync.dma_start(dbg_partial[:], partial[:])

            # ---- phase D: masked dense GEMM ----
            for c in range(NSCH):
                psums = [
                    psp.tile([DW, SCH], dt.float32, tag=f"ps{d}", name=f"ps_c{c}_d{d}")
                    for d in range(DT)
                ]
                for t in range(JT):
                    xtile = xtp.tile([128, SCH], mmdt)
                    nc.sync.dma_start(
                        xtile[:], xt[t * 128 : (t + 1) * 128, c * SCH : (c + 1) * SCH]
                    )
                    wtile = wtp.tile([128, DSH], mmdt)
                    nc.sync.dma_start(wtile[:], wt[t * 128 : (t + 1) * 128, :])
                    wmt = wmp.tile([128, DSH], mmdt)
                    nc.vector.tensor_scalar(
                        out=wmt[:],
                        in0=wtile[:],
                        scalar1=mask[:, t : t + 1],
                        scalar2=None,
                        op0=Alu.mult,
                    )
                    for d in range(DT):
                        nc.tensor.matmul(
                            psums[d][:],
                            lhsT=wmt[:, d * DW : (d + 1) * DW],
                            rhs=xtile[:],
                            start=(t == 0),
                            stop=(t == JT - 1),
                        )
                for d in range(DT):
                    ot = otp.tile([DW, SCH], dt.float32)
                    nc.scalar.copy(ot[:], psums[d][:])
                    nc.sync.dma_start(
                        outT[d * DW : (d + 1) * DW, c * SCH : (c + 1) * SCH], ot[:]
                    )

    nc.compile()
    return nc


def _get_program(cfg):
    key = (cfg["name"], cfg.get("mm_dtype", MM_DTYPE))
    if key not in _cache:
        _cache[key] = _build_program(cfg)
    return _cache[key]


def _stage_inputs(x, W, cfg):
    """Host-side sharding/layout. Returns per-core in_maps."""
    DFF = cfg["dff"]
    S = cfg["s"]
    D = cfg["d"]
    DSH = D // N_CORES
    SSH = S // N_CORES

    x2d = np.ascontiguousarray(np.asarray(x, dtype=np.float32).reshape(S, DFF))
    Wf = np.asarray(W, dtype=np.float32)

    xT = np.ascontiguousarray(x2d.T)          # [DFF, S]
    WT = np.ascontiguousarray(Wf.T)           # [DFF, D]

    if cfg.get("mm_dtype", MM_DTYPE) == "f32":
        xT_mm = xT
        WT_mm = WT
    else:
        import ml_dtypes

        xT_mm = xT.astype(ml_dtypes.bfloat16)
        WT_mm = WT.astype(ml_dtypes.bfloat16)

    in_maps = []
    for c in range(N_CORES):
        in_maps.append(
            {
                "xs": np.ascontiguousarray(xT[:, c * SSH : (c + 1) * SSH]),
                "xt": xT_mm,
                "wt": np.ascontiguousarray(WT_mm[:, c * DSH : (c + 1) * DSH]),
            }
        )
    return in_maps


def run_cfg(x, W, cfg, trace=False, trace_kwargs=None):
    """Run the kernel for a given cfg; returns (out, BassKernelResults)."""
    from concourse.bass_utils import run_bass_kernel_spmd

    S, D = cfg["s"], cfg["d"]
    DSH = D // N_CORES
    nc = _get_program(cfg)
    in_maps = _stage_inputs(x, W, cfg)
    res = run_bass_kernel_spmd(
        nc,
        in_maps,
        core_ids=list(range(N_CORES)),
        trace=trace,
        **(trace_kwargs or {}),
    )
    outT = np.concatenate([res.results[c]["outT"] for c in range(N_CORES)], axis=0)
    out = np.ascontiguousarray(outT.T).reshape(1, S, D).astype(np.float32)
    return out, res


def kernel(x, W):
    out, _ = run_cfg(x, W, FULL_CFG)
    return out



# revision 8
# speedup vs baseline: 2.2473x; 2.2473x over previous
"""Trainium2 Bass kernel for nn_CustomMLPLayer_13408887898971 (topk_masking).

Computes (matching reference.py):
    scores = sum_s relu(x[0,s,:])          # [d_ff]
    idx    = top_k(scores, K)              # K = 4403
    out    = x[..., idx] @ W[:, idx].T     # [1, S, d_model]

Key identity: gathering the same top-K columns of x and W and contracting
equals a dense contraction with the non-top-K columns masked to zero:
    out = (x * m) @ W.T  with  m[j] = scores[j] >= (K-th largest score)

Strategy (8 NeuronCores, tensor-parallel over d_model):
  - host: transpose x and W to j-major (contraction on partitions),
    shard W.T by d_model columns (512 per core), x.T replicated, both
    cast to bf16 for the GEMM (psum accumulates f32; output rel err
    ~3e-3, far inside the 2e-2 gate). Scores use the f32 x.T shard.
  - device, per core:
      phase A: partial scores over this core's 256-token shard.
               Exactness trick (selection must be exact: one swapped
               boundary neuron alone costs ~2e-2 rel err): two-limb
               accumulation of r = relu(x)*1024 — integer part h sums
               exactly in f32, fractional residue r1 sums with ~1e-6
               noise. Engine split: ACT does relu and the +2^23
               round-trip bias pass; DVE does the two accumulating
               passes. W tiles DMA into SBUF concurrently (resident).
      phase B: AllReduce partial scores across the 8 cores (88KB)
      phase C: exact K-th largest via radix-16 binary search on the f32
               bit pattern (non-negative floats order like ints); then
               mask = scores >= threshold
      phase D: masked dense GEMM from SBUF-resident W: mask W in place
               (86 ops), then psum[d,s] += Wm[jt].T @ x.T[jt,s]
               accumulated over 86 j-tiles, streaming x.T bf16 tiles.
  - host: concat per-core [512, 2048] out.T shards, transpose.
"""

import numpy as np

N_CORES = 8

FULL_CFG = dict(
    dff=11008,
    s=2048,
    d=4096,
    k=4403,
    name="full",
)

# matmul operand dtype: "f32" (exact, 4 cyc/row) or "bf16" (1 cyc/row)
MM_DTYPE = "bf16"

_cache = {}


def _build_program(cfg):
    """Build + compile the 8-core SPMD bass program. Returns nc."""
    from concourse import bacc, tile
    import concourse.bass as bass
    import concourse.mybir as mybir
    import concourse.bass_isa as bass_isa

    dt = mybir.dt
    Alu = mybir.AluOpType

    DFF = cfg["dff"]
    S = cfg["s"]
    D = cfg["d"]
    K = cfg["k"]
    DSH = D // N_CORES           # d_model cols per core
    SSH = S // N_CORES           # score-token shard per core
    JT = DFF // 128              # j tiles
    SCH = min(512, S)            # moving free dim per matmul
    NSCH = S // SCH              # s chunks
    DT = max(1, DSH // 128)      # d tiles per core (lhsT free dim 128)
    assert DSH % 128 == 0 or DSH < 128
    DW = min(128, DSH)           # width of a d tile

    mmdt = dt.float32 if cfg.get("mm_dtype", MM_DTYPE) == "f32" else dt.bfloat16

    nc = bacc.Bacc(
        "TRN2", target_bir_lowering=False, debug=False, num_devices=N_CORES
    )

    # I/O (per-core tensors; in_maps provide per-core data)
    xs = nc.dram_tensor("xs", [DFF, SSH], dt.float32, kind="ExternalInput").ap()
    xt = nc.dram_tensor("xt", [DFF, S], mmdt, kind="ExternalInput").ap()
    wt = nc.dram_tensor("wt", [DFF, DSH], mmdt, kind="ExternalInput").ap()
    outT = nc.dram_tensor("outT", [DSH, S], dt.float32, kind="ExternalOutput").ap()

    with tile.TileContext(nc) as tc:
        with (
            tc.tile_pool(name="persist", bufs=1) as pp,
            tc.tile_pool(name="xs_p", bufs=3) as xsp,
            tc.tile_pool(name="relu_p", bufs=3) as rlp,
            tc.tile_pool(name="xt_p", bufs=4) as xtp,
            tc.tile_pool(name="out_p", bufs=3) as otp,
            tc.tile_pool(name="psum", bufs=2, space="PSUM") as psp,
            tc.tile_pool(name="dram", bufs=1, space="DRAM") as drp,
        ):
            # ---- persistent small tiles ----
            # partial holds [hsum | rsum]: integer-part sums (exact in f32)
            # and fractional-residue sums of relu(x)*1024 per j.
            partial = pp.tile([128, 2 * JT], dt.float32, tag="partial")
            scores = pp.tile([128, JT], dt.float32, tag="scores")
            mask = pp.tile([128, JT], dt.float32, tag="mask")
            thr = pp.tile([128, 1], dt.int32, tag="thr")
            cand = pp.tile([128, 1], dt.int32, tag="cand")
            ge_scr = pp.tile([128, JT], dt.float32, tag="ge_scr")
            cnts = pp.tile([128, 15], dt.float32, tag="cnts")
            cntr = pp.tile([128, 15], dt.float32, tag="cntr")
            sel = pp.tile([128, 15], dt.float32, tag="sel")
            digf = pp.tile([128, 1], dt.float32, tag="digf")
            digi = pp.tile([128, 1], dt.int32, tag="digi")

            # W resident in SBUF: tile t at columns [t*DSH, (t+1)*DSH)
            wres = pp.tile([128, JT * DSH], mmdt, tag="wres")
            for t in range(JT):
                nc.sync.dma_start(
                    wres[:, t * DSH : (t + 1) * DSH],
                    wt[t * 128 : (t + 1) * 128, :],
                )

            # ---- phase A: partial scores over this core's token shard ----
            # Engine split: ACT computes r = relu(1024*x) and later reduces
            # r1 (accum_out on an Identity pass); DVE computes
            # h = (r + 2^23) - 2^23 in one fused two-op pass (exact f32
            # round-to-int for r < 2^13, accumulating hsum) and r1 = r - h.
            # |r1| <= 0.5, h integer, so hsum is exact and rsum noise ~1e-6.
            for t in range(JT):
                st = xsp.tile([128, SSH], dt.float32)
                nc.sync.dma_start(st[:], xs[t * 128 : (t + 1) * 128, :])
                rt = rlp.tile([128, SSH], dt.float32, tag="rt")
                nc.scalar.activation(
                    rt[:],
                    st[:],
                    mybir.ActivationFunctionType.Relu,
                    scale=1024.0,
                )
                tmpt = rlp.tile([128, SSH], dt.float32, tag="tmpt")
                nc.vector.tensor_scalar(
                    out=tmpt[:],
                    in0=rt[:],
                    scalar1=float(2.0**23),
                    scalar2=None,
                    op0=Alu.add,
                )
                ht = rlp.tile([128, SSH], dt.float32, tag="ht")
                nc.vector.tensor_scalar(
                    out=ht[:],
                    in0=tmpt[:],
                    scalar1=float(2.0**23),
                    scalar2=0.0,
                    op0=Alu.subtract,
                    op1=Alu.add,
                    accum_out=partial[:, t : t + 1],
                )
                r1t = rlp.tile([128, SSH], dt.float32, tag="r1t")
                nc.vector.tensor_tensor(
                    out=r1t[:], in0=rt[:], in1=ht[:], op=Alu.subtract
                )
                r1c = rlp.tile([128, SSH], dt.float32, tag="r1c")
                nc.scalar.activation(
                    r1c[:],
                    r1t[:],
                    mybir.ActivationFunctionType.Identity,
                    accum_out=partial[:, JT + t : JT + t + 1],
                )

            # ---- phase B: AllReduce partial sums across cores ----
            cc_in = drp.tile([128, 2 * JT], dt.float32)
            cc_out = drp.tile([128, 2 * JT], dt.float32)
            nc.sync.dma_start(cc_in[:], partial[:])
            nc.gpsimd.collective_compute(
                "AllReduce",
                Alu.add,
                replica_groups=[list(range(N_CORES))],
                ins=[cc_in.opt()],
                outs=[cc_out.opt()],
            )
            hr = pp.tile([128, 2 * JT], dt.float32, tag="hr")
            nc.sync.dma_start(hr[:], cc_out[:])
            # scores = (hsum + rsum) * 2^-10  (single final rounding)
            nc.vector.tensor_tensor(
                out=scores[:], in0=hr[:, :JT], in1=hr[:, JT:], op=Alu.add
            )
            nc.vector.tensor_scalar(
                out=scores[:],
                in0=scores[:],
                scalar1=float(2.0**-10),
                scalar2=None,
                op0=Alu.mult,
            )

            # ---- phase C: K-th largest via radix-16 search on f32 bits ----
            # scores >= 0, so f32 bit patterns order like int32. Candidates are
            # built in int32 bit space, and compared in f32 space (bitcast the
            # candidate): order is identical for non-negative values, and
            # candidates that land in the inf/nan range compare as "no score
            # >= cand", matching the int compare.
            # NOTE: the DVE ALU evaluates int32 tensor ops in f32 arithmetic,
            # so bit-space increments below ULP(thr_bits ~ 2^30) = 128 are
            # rounded away. The int-bit stage therefore only resolves bits
            # 7..30 (increments are multiples of 128 -> exact in f32); the
            # low 7 bits are resolved in float space using exact ULP steps.
            nc.vector.memset(thr[:], 0)

            def count_round(make_cand, ncand, upd):
                """One radix round: count candidates, pick digit, update thr."""
                for r in range(1, ncand + 1):
                    make_cand(r)
                    nc.vector.tensor_scalar(
                        out=ge_scr[:],
                        in0=scores[:],
                        scalar1=candf[:],
                        scalar2=0.0,
                        op0=Alu.is_ge,
                        op1=Alu.add,
                        accum_out=cnts[:, r - 1 : r],
                    )
                nc.gpsimd.partition_all_reduce(
                    cntr[:, :ncand],
                    cnts[:, :ncand],
                    channels=128,
                    reduce_op=bass_isa.ReduceOp.add,
                )
                nc.vector.tensor_scalar(
                    out=sel[:, :ncand],
                    in0=cntr[:, :ncand],
                    scalar1=float(K),
                    scalar2=None,
                    op0=Alu.is_ge,
                )
                nc.vector.tensor_reduce(
                    out=digf[:],
                    in_=sel[:, :ncand],
                    axis=mybir.AxisListType.X,
                    op=Alu.add,
                )
                upd()

            candf = pp.tile([128, 1], dt.float32, tag="candf")
            thr_f = pp.tile([128, 1], dt.float32, tag="thr_f")
            ulp = pp.tile([128, 1], dt.float32, tag="ulp")
            step = pp.tile([128, 1], dt.float32, tag="step")

            # --- int-bit stage: bits 7..30, radix 16 ---
            for shift in (27, 23, 19, 15, 11, 7):

                def make_cand_int(r, shift=shift):
                    nc.vector.tensor_scalar(
                        out=cand[:],
                        in0=thr[:],
                        scalar1=r << shift,
                        scalar2=None,
                        op0=Alu.add,
                    )
                    # view the int candidate as f32 for the compare
                    nc.vector.tensor_scalar(
                        out=candf[:],
                        in0=cand[:].bitcast(dt.float32),
                        scalar1=0.0,
                        scalar2=None,
                        op0=Alu.add,
                    )

                def upd_int(shift=shift):
                    nc.vector.tensor_scalar(
                        out=digi[:],
                        in0=digf[:],
                        scalar1=float(1 << shift),
                        scalar2=None,
                        op0=Alu.mult,
                    )
                    nc.vector.tensor_tensor(
                        out=thr[:], in0=thr[:], in1=digi[:], op=Alu.add
                    )

                count_round(make_cand_int, 15, upd_int)

            # --- float stage: low 7 bits with exact ULP steps ---
            # ulp = (bitcast(thr+128) - bitcast(thr)) / 128 (exact powers of 2)
            nc.vector.tensor_scalar(
                out=cand[:], in0=thr[:], scalar1=128, scalar2=None, op0=Alu.add
            )
            nc.vector.tensor_tensor(
                out=ulp[:],
                in0=cand[:].bitcast(dt.float32),
                in1=thr[:].bitcast(dt.float32),
                op=Alu.subtract,
            )
            nc.vector.tensor_scalar(
                out=ulp[:],
                in0=ulp[:],
                scalar1=1.0 / 128.0,
                scalar2=None,
                op0=Alu.mult,
            )
            nc.vector.tensor_scalar(
                out=thr_f[:],
                in0=thr[:].bitcast(dt.float32),
                scalar1=0.0,
                scalar2=None,
                op0=Alu.add,
            )

            for mult_, ncand in ((16, 7), (1, 15)):

                def make_cand_f(r, mult_=mult_):
                    nc.vector.tensor_scalar(
                        out=step[:],
                        in0=ulp[:],
                        scalar1=float(r * mult_),
                        scalar2=None,
                        op0=Alu.mult,
                    )
                    nc.vector.tensor_tensor(
                        out=candf[:], in0=thr_f[:], in1=step[:], op=Alu.add
                    )

                def upd_f(mult_=mult_):
                    nc.vector.tensor_scalar(
                        out=digf[:],
                        in0=digf[:],
                        scalar1=float(mult_),
                        scalar2=None,
                        op0=Alu.mult,
                    )
                    nc.vector.tensor_tensor(
                        out=step[:], in0=digf[:], in1=ulp[:], op=Alu.mult
                    )
                    nc.vector.tensor_tensor(
                        out=thr_f[:], in0=thr_f[:], in1=step[:], op=Alu.add
                    )

                count_round(make_cand_f, ncand, upd_f)

            # mask[j] = scores >= thr_f  (0.0/1.0 f32)
            nc.vector.tensor_scalar(
                out=mask[:],
                in0=scores[:],
                scalar1=thr_f[:],
                scalar2=None,
                op0=Alu.is_ge,
            )

            # ---- phase D: masked dense GEMM from resident W ----
            # mask W in place (per-partition scalar broadcast along free)
            for t in range(JT):
                nc.vector.tensor_scalar(
                    out=wres[:, t * DSH : (t + 1) * DSH],
                    in0=wres[:, t * DSH : (t + 1) * DSH],
                    scalar1=mask[:, t : t + 1],
                    scalar2=None,
                    op0=Alu.mult,
                )

            for c in range(NSCH):
                psums = [
                    psp.tile([DW, SCH], dt.float32, tag=f"ps{d}", name=f"ps_c{c}_d{d}")
                    for d in range(DT)
                ]
                for t in range(JT):
                    xtile = xtp.tile([128, SCH], mmdt)
                    nc.sync.dma_start(
                        xtile[:], xt[t * 128 : (t + 1) * 128, c * SCH : (c + 1) * SCH]
                    )
                    for d in range(DT):
                        nc.tensor.matmul(
                            psums[d][:],
                            lhsT=wres[:, t * DSH + d * DW : t * DSH + (d + 1) * DW],
                            rhs=xtile[:],
                            start=(t == 0),
                            stop=(t == JT - 1),
                        )
                for d in range(DT):
                    ot = otp.tile([DW, SCH], dt.float32)
                    nc.scalar.copy(ot[:], psums[d][:])
                    nc.sync.dma_start(
                        outT[d * DW : (d + 1) * DW, c * SCH : (c + 1) * SCH], ot[:]
                    )

    nc.compile()
    return nc


def _get_program(cfg):
    key = (cfg["name"], cfg.get("mm_dtype", MM_DTYPE))
    if key not in _cache:
        _cache[key] = _build_program(cfg)
    return _cache[key]


def _stage_inputs(x, W, cfg):
    """Host-side sharding/layout. Returns per-core in_maps."""
    DFF = cfg["dff"]
    S = cfg["s"]
    D = cfg["d"]
    DSH = D // N_CORES
    SSH = S // N_CORES

    x2d = np.ascontiguousarray(np.asarray(x, dtype=np.float32).reshape(S, DFF))
    Wf = np.asarray(W, dtype=np.float32)

    xT = np.ascontiguousarray(x2d.T)          # [DFF, S]
    WT = np.ascontiguousarray(Wf.T)           # [DFF, D]

    if cfg.get("mm_dtype", MM_DTYPE) == "f32":
        xT_mm = xT
        WT_mm = WT
    else:
        import ml_dtypes

        xT_mm = xT.astype(ml_dtypes.bfloat16)
        WT_mm = WT.astype(ml_dtypes.bfloat16)

    in_maps = []
    for c in range(N_CORES):
        in_maps.append(
            {
                "xs": np.ascontiguousarray(xT[:, c * SSH : (c + 1) * SSH]),
                "xt": xT_mm,
                "wt": np.ascontiguousarray(WT_mm[:, c * DSH : (c + 1) * DSH]),
            }
        )
    return in_maps


def run_cfg(x, W, cfg, trace=False, trace_kwargs=None):
    """Run the kernel for a given cfg; returns (out, BassKernelResults)."""
    from concourse.bass_utils import run_bass_kernel_spmd

    S, D = cfg["s"], cfg["d"]
    DSH = D // N_CORES
    nc = _get_program(cfg)
    in_maps = _stage_inputs(x, W, cfg)
    res = run_bass_kernel_spmd(
        nc,
        in_maps,
        core_ids=list(range(N_CORES)),
        trace=trace,
        **(trace_kwargs or {}),
    )
    outT = np.concatenate([res.results[c]["outT"] for c in range(N_CORES)], axis=0)
    out = np.ascontiguousarray(outT.T).reshape(1, S, D).astype(np.float32)
    return out, res


def kernel(x, W):
    out, _ = run_cfg(x, W, FULL_CFG)
    return out


# revision 17
# speedup vs baseline: 3.6099x; 1.6063x over previous
"""Trainium2 Bass kernel for nn_CustomMLPLayer_13408887898971 (topk_masking).

Computes (matching reference.py):
    scores = sum_s relu(x[0,s,:])          # [d_ff]
    idx    = top_k(scores, K)              # K = 4403
    out    = x[..., idx] @ W[:, idx].T     # [1, S, d_model]

Strategy (8 NeuronCores, tensor-parallel over d_model), fully sparse:
  - host: transpose x and W to j-major, shard W.T by d_model columns
    (512 per core), x.T replicated; both cast to bf16 with one extra
    ZERO row appended (row DFF) — the gather pad target. Scores use the
    f32 x.T token-shard. A [128,128] strictly-lower-triangular ones
    matrix is staged for the on-device partition prefix sum.
  - device, per core:
      A: partial scores over the 256-token shard. Exact two-limb
         accumulation of r = relu(x)*1024 (integer limb sums exactly in
         f32; fractional residue sums with ~1e-6 noise) — selection
         must be exact: one swapped boundary neuron alone costs ~2e-2
         rel err. Chunked big ops: ACT relu, DVE round-trick +
         segmented reduces, GPSIMD the r1 subtract.
      B: AllReduce partial sums across the 8 cores (88KB).
      C: exact K-th largest via radix-16 search on the f32 bit pattern
         (non-negative floats order like ints). Counting is batched:
         one [128,15,86] broadcast-compare + segmented reduce per
         round instead of 15 small ops.
      D: sparse gather GEMM. Build a compacted slot->index list of the
         selected neurons ON DEVICE: rank = (#selected in lower
         partitions, via matmul with the strict-lower-tri ones) +
         (free-axis exclusive cumsum within the partition row); a
         permutation matmul scatters j indices into list slots; pad
         slots point at the appended zero W row. Then 35 indirect-DMA
         row gathers of W.T (-> SBUF resident) and 70 of x.T halves,
         and the GEMM contracts only 35 j-tiles instead of 86:
         psum[d,s] += Wg[u].T @ xg[u,s].
  - host: concat per-core [512, 2048] out.T shards, transpose.
"""

import numpy as np

N_CORES = 8

FULL_CFG = dict(
    dff=11008,
    s=2048,
    d=4096,
    k=4403,
    name="full",
)

MM_DTYPE = "bf16"
BIG = float(1 << 20)  # rank offset for unselected entries

_cache = {}


def _build_program(cfg):
    """Build + compile the 8-core SPMD bass program. Returns nc."""
    from concourse import bacc, tile
    import concourse.bass as bass
    import concourse.mybir as mybir
    import concourse.bass_isa as bass_isa

    dt = mybir.dt
    Alu = mybir.AluOpType
    AF = mybir.ActivationFunctionType

    DFF = cfg["dff"]
    S = cfg["s"]
    D = cfg["d"]
    K = cfg["k"]
    DSH = D // N_CORES           # d_model cols per core
    SSH = S // N_CORES           # score-token shard per core
    JT = DFF // 128              # j tiles
    SCH = min(512, S)            # psum free dim per matmul
    NSCH = S // SCH              # s chunks
    DT = max(1, DSH // 128)      # d tiles per core
    DW = min(128, DSH)
    U = K // 128 + 1             # gathered slot tiles (capacity U*128 >= K)
    SG = min(2 * SCH, S)         # token-group width per xg gather
    CL = SG // SCH               # chunks per group
    NG = S // SG                 # groups
    GA = 4                       # phase-A tiles per chunked op
    assert DSH % 128 == 0 or DSH < 128
    assert CL * DT <= 8, "psum banks"

    mmdt = dt.float32 if cfg.get("mm_dtype", MM_DTYPE) == "f32" else dt.bfloat16

    nc = bacc.Bacc(
        "TRN2", target_bir_lowering=False, debug=False, num_devices=N_CORES,
        num_swdge_queues=4,
    )

    # I/O (per-core tensors; in_maps provide per-core data)
    xs = nc.dram_tensor("xs", [DFF, SSH], dt.float32, kind="ExternalInput").ap()
    xt = nc.dram_tensor("xt", [DFF + 1, S], mmdt, kind="ExternalInput").ap()
    wt = nc.dram_tensor("wt", [DFF + 1, DSH], mmdt, kind="ExternalInput").ap()
    lst = nc.dram_tensor("lst", [128, 128], dt.float32, kind="ExternalInput").ap()
    outT = nc.dram_tensor("outT", [DSH, S], dt.float32, kind="ExternalOutput").ap()
    debug = cfg.get("debug", False)
    if debug:
        dbg = {
            name: nc.dram_tensor(f"dbg_{name}", [128, w], dty, kind="ExternalOutput").ap()
            for name, w, dty in [
                ("scores", JT, dt.float32),
                ("mask", JT, dt.float32),
                ("rank", JT, dt.float32),
                ("rkeff", JT, dt.float32),
                ("div", JT, dt.float32),
                ("rmod", JT, dt.float32),
                ("list", U, dt.float32),
                ("listi", U, dt.int32),
                ("cnt", 1, dt.float32),
                ("poff", 1, dt.float32),
                ("npc", 1, dt.float32),
                ("thrf", 1, dt.float32),
                ("wg", U * DSH, dt.float32),
                ("dsel", JT * U, dt.float32),
                ("rhsall", JT * U, dt.float32),
                ("i128", 128, dt.int32),
                ("jval", JT, dt.int32),
                ("lt0", 128, dt.float32),
            ]
        }

    with tile.TileContext(nc) as tc:
        with (
            tc.tile_pool(name="persist", bufs=1) as pp,
            tc.tile_pool(name="xs_p", bufs=2) as xsp,
            tc.tile_pool(name="relu_p", bufs=2) as rlp,
            tc.tile_pool(name="lhsT_p", bufs=2) as ltp,
            tc.tile_pool(name="xg_p", bufs=3) as xgp,
            tc.tile_pool(name="out_p", bufs=3) as otp,
            tc.tile_pool(name="psum", bufs=1, space="PSUM") as psp,
            tc.tile_pool(name="dram", bufs=1, space="DRAM") as drp,
        ):
            # ---- persistent small tiles ----
            partial = pp.tile([128, 2 * JT], dt.float32, tag="partial")
            scores = pp.tile([128, JT], dt.float32, tag="scores")
            mask = pp.tile([128, JT], dt.float32, tag="mask")
            ge3 = pp.tile([128, 15 * JT], dt.float32, tag="ge3")
            cnts = pp.tile([128, 15], dt.float32, tag="cnts")
            cntr = pp.tile([128, 15], dt.float32, tag="cntr")
            sel = pp.tile([128, 15], dt.float32, tag="sel")
            digf = pp.tile([128, 1], dt.float32, tag="digf")
            thrv = pp.tile([128, 1], dt.float32, tag="thrv")
            thri = pp.tile([128, 1], dt.int32, tag="thri")
            candi15 = pp.tile([128, 15], dt.int32, tag="candi15")
            candv15 = pp.tile([128, 15], dt.float32, tag="candv15")
            candf15 = pp.tile([128, 15], dt.float32, tag="candf15")
            cand = pp.tile([128, 1], dt.int32, tag="cand")
            thr_f = pp.tile([128, 1], dt.float32, tag="thr_f")
            ulp = pp.tile([128, 1], dt.float32, tag="ulp")
            s15 = pp.tile([128, 15], dt.float32, tag="s15")
            hr = pp.tile([128, 2 * JT], dt.float32, tag="hr")

            # sparse-gather persistents
            wg = pp.tile([128, U * DSH], mmdt, tag="wg")
            lst_sb = pp.tile([128, 128], dt.float32, tag="lst_sb")
            i128 = pp.tile([128, 128], dt.int32, tag="i128")
            i35 = pp.tile([128, U], dt.int32, tag="i35")
            i15 = pp.tile([128, 15], dt.int32, tag="i15")
            jval = pp.tile([128, JT], dt.int32, tag="jval")
            islot = pp.tile([128, U], dt.int32, tag="islot")
            npc = pp.tile([128, 1], dt.float32, tag="npc")
            poff = pp.tile([128, 1], dt.float32, tag="poff")
            ca = pp.tile([128, JT], dt.float32, tag="ca")
            cb = pp.tile([128, JT], dt.float32, tag="cb")
            rank = pp.tile([128, JT], dt.float32, tag="rank")
            rkeff = pp.tile([128, JT], dt.float32, tag="rkeff")
            im = pp.tile([128, JT], dt.float32, tag="im")
            div = pp.tile([128, JT], dt.float32, tag="div")
            rmod = pp.tile([128, JT], dt.float32, tag="rmod")
            dsel = pp.tile([128, JT * U], dt.float32, tag="dsel")
            rhs_all = pp.tile([128, JT * U], dt.float32, tag="rhs_all")
            cnt_col = pp.tile([128, 1], dt.float32, tag="cnt_col")
            cnt_all = pp.tile([128, 1], dt.float32, tag="cnt_all")
            padi = pp.tile([128, U], dt.float32, tag="padi")
            list_sb = pp.tile([128, U], dt.float32, tag="list_sb")
            list_i32 = pp.tile([128, U], dt.int32, tag="list_i32")

            nc.sync.dma_start(lst_sb[:], lst[:])
            nc.gpsimd.iota(i128[:], pattern=[[1, 128]], base=0, channel_multiplier=0)
            nc.gpsimd.iota(i35[:], pattern=[[1, U]], base=0, channel_multiplier=0)
            nc.gpsimd.iota(i15[:], pattern=[[1, 15]], base=1, channel_multiplier=0)
            nc.gpsimd.iota(jval[:], pattern=[[128, JT]], base=0, channel_multiplier=1)
            nc.gpsimd.iota(islot[:], pattern=[[128, U]], base=0, channel_multiplier=1)

            # ---- phase A: partial scores over this core's token shard ----
            t0 = 0
            while t0 < JT:
                G = min(GA, JT - t0)
                W_ = G * SSH
                st = xsp.tile([128, GA * SSH], dt.float32, tag="st")
                for g in range(G):
                    nc.sync.dma_start(
                        st[:, g * SSH : (g + 1) * SSH],
                        xs[(t0 + g) * 128 : (t0 + g + 1) * 128, :],
                    )
                rt = rlp.tile([128, GA * SSH], dt.float32, tag="rt")
                nc.scalar.activation(rt[:, :W_], st[:, :W_], AF.Relu, scale=1024.0)
                tmpt = rlp.tile([128, GA * SSH], dt.float32, tag="tmpt")
                nc.vector.tensor_scalar(
                    out=tmpt[:, :W_], in0=rt[:, :W_],
                    scalar1=float(2.0**23), scalar2=None, op0=Alu.add,
                )
                ht = rlp.tile([128, GA * SSH], dt.float32, tag="ht")
                nc.vector.tensor_scalar(
                    out=ht[:, :W_], in0=tmpt[:, :W_],
                    scalar1=float(2.0**23), scalar2=None, op0=Alu.subtract,
                )
                nc.vector.tensor_reduce(
                    out=partial[:, t0 : t0 + G],
                    in_=ht[:, :W_].rearrange("p (g s) -> p g s", g=G),
                    axis=mybir.AxisListType.X, op=Alu.add,
                )
                r1t = rlp.tile([128, GA * SSH], dt.float32, tag="r1t")
                nc.gpsimd.tensor_tensor(
                    out=r1t[:, :W_], in0=rt[:, :W_], in1=ht[:, :W_], op=Alu.subtract
                )
                nc.vector.tensor_reduce(
                    out=partial[:, JT + t0 : JT + t0 + G],
                    in_=r1t[:, :W_].rearrange("p (g s) -> p g s", g=G),
                    axis=mybir.AxisListType.X, op=Alu.add,
                )
                t0 += G

            # ---- phase B: AllReduce partial sums across cores ----
            cc_in = drp.tile([128, 2 * JT], dt.float32)
            cc_out = drp.tile([128, 2 * JT], dt.float32)
            nc.sync.dma_start(cc_in[:], partial[:])
            nc.gpsimd.collective_compute(
                "AllReduce",
                Alu.add,
                replica_groups=[list(range(N_CORES))],
                ins=[cc_in.opt()],
                outs=[cc_out.opt()],
            )
            nc.sync.dma_start(hr[:], cc_out[:])
            nc.vector.tensor_tensor(
                out=scores[:], in0=hr[:, :JT], in1=hr[:, JT:], op=Alu.add
            )
            nc.vector.tensor_scalar(
                out=scores[:], in0=scores[:],
                scalar1=float(2.0**-10), scalar2=None, op0=Alu.mult,
            )

            # ---- phase C: K-th largest via radix-16 search on f32 bits ----
            # thrv holds the (non-negative) candidate bit pattern as an f32
            # VALUE; all increments are multiples of 2^7 so f32 arithmetic on
            # the bit values is exact. The low 7 bits are resolved in float
            # space with exact ULP steps. Counting is batched: candidates
            # r=1..15 compared against all scores in one [128,15,JT] op.
            nc.vector.memset(thrv[:], 0.0)

            def count_batched(cand_ap, ncand):
                """counts[r] = #{j: scores[j] >= cand[r]} for r in [0,ncand)."""
                nc.vector.tensor_tensor(
                    out=ge3[:, : ncand * JT].rearrange("p (r t) -> p r t", t=JT),
                    in0=scores[:].unsqueeze(1).broadcast_to([128, ncand, JT]),
                    in1=cand_ap.to_broadcast([128, ncand, JT]),
                    op=Alu.is_ge,
                )
                nc.vector.tensor_reduce(
                    out=cnts[:, :ncand],
                    in_=ge3[:, : ncand * JT].rearrange("p (r t) -> p r t", t=JT),
                    axis=mybir.AxisListType.X, op=Alu.add,
                )
                nc.gpsimd.partition_all_reduce(
                    cntr[:, :ncand], cnts[:, :ncand],
                    channels=128, reduce_op=bass_isa.ReduceOp.add,
                )
                nc.vector.tensor_scalar(
                    out=sel[:, :ncand], in0=cntr[:, :ncand],
                    scalar1=float(K), scalar2=None, op0=Alu.is_ge,
                )
                nc.vector.tensor_reduce(
                    out=digf[:], in_=sel[:, :ncand],
                    axis=mybir.AxisListType.X, op=Alu.add,
                )

            # --- int-bit stage: bits 7..30, radix 16 ---
            for shift in (27, 23, 19, 15, 11, 7):
                nc.vector.tensor_scalar(
                    out=candv15[:], in0=i15[:],
                    scalar1=float(1 << shift), scalar2=thrv[:],
                    op0=Alu.mult, op1=Alu.add,
                )
                nc.vector.tensor_copy(out=candi15[:], in_=candv15[:])
                count_batched(candi15[:].bitcast(dt.float32), 15)
                nc.vector.tensor_scalar(
                    out=digf[:], in0=digf[:],
                    scalar1=float(1 << shift), scalar2=None, op0=Alu.mult,
                )
                nc.vector.tensor_tensor(
                    out=thrv[:], in0=thrv[:], in1=digf[:], op=Alu.add
                )

            # --- float stage: low 7 bits with exact ULP steps ---
            nc.vector.tensor_copy(out=thri[:], in_=thrv[:])
            nc.vector.tensor_scalar(
                out=cand[:], in0=thri[:], scalar1=128, scalar2=None, op0=Alu.add
            )
            nc.vector.tensor_tensor(
                out=ulp[:],
                in0=cand[:].bitcast(dt.float32),
                in1=thri[:].bitcast(dt.float32),
                op=Alu.subtract,
            )
            nc.vector.tensor_scalar(
                out=ulp[:], in0=ulp[:],
                scalar1=1.0 / 128.0, scalar2=None, op0=Alu.mult,
            )
            nc.vector.tensor_scalar(
                out=thr_f[:], in0=thri[:].bitcast(dt.float32),
                scalar1=0.0, scalar2=None, op0=Alu.add,
            )

            for mult_, ncand in ((16, 7), (1, 15)):
                nc.vector.tensor_scalar(
                    out=s15[:, :ncand], in0=i15[:, :ncand],
                    scalar1=float(mult_), scalar2=ulp[:], op0=Alu.mult, op1=Alu.mult,
                )
                nc.vector.tensor_scalar(
                    out=candf15[:, :ncand], in0=s15[:, :ncand],
                    scalar1=thr_f[:], scalar2=None, op0=Alu.add,
                )
                count_batched(candf15[:, :ncand], ncand)
                nc.vector.tensor_scalar(
                    out=digf[:], in0=digf[:],
                    scalar1=float(mult_), scalar2=None, op0=Alu.mult,
                )
                nc.vector.tensor_tensor(
                    out=s15[:, :1], in0=digf[:], in1=ulp[:], op=Alu.mult
                )
                nc.vector.tensor_tensor(
                    out=thr_f[:], in0=thr_f[:], in1=s15[:, :1], op=Alu.add
                )

            # mask[j] = scores >= thr_f  (0.0/1.0 f32)
            nc.vector.tensor_scalar(
                out=mask[:], in0=scores[:],
                scalar1=thr_f[:], scalar2=None, op0=Alu.is_ge,
            )

            # ---- phase D1: build the compacted slot -> j index list ----
            # rank[p,t] = #selected with (p' < p) + #selected in row p with
            # t' < t; bijective onto [0, cnt). Slot (q, u) holds the j with
            # rank u*128 + q; pad slots (rank >= cnt) point at row DFF
            # (zero W row => zero contribution).
            nc.vector.tensor_reduce(
                out=npc[:], in_=mask[:], axis=mybir.AxisListType.X, op=Alu.add
            )
            poff_ps = psp.tile([128, 1], dt.float32, tag="ps1", name="poff_ps")
            nc.tensor.matmul(poff_ps[:], lhsT=lst_sb[:], rhs=npc[:], start=True, stop=True)
            nc.vector.tensor_copy(out=poff[:], in_=poff_ps[:])
            # inclusive cumsum of mask along t (log-shift ping-pong)
            nc.vector.tensor_copy(out=ca[:], in_=mask[:])
            src, dst = ca, cb
            sh = 1
            while sh < JT:
                nc.vector.tensor_copy(out=dst[:, :sh], in_=src[:, :sh])
                nc.vector.tensor_tensor(
                    out=dst[:, sh:], in0=src[:, sh:], in1=src[:, : JT - sh], op=Alu.add
                )
                src, dst = dst, src
                sh *= 2
            # rank = (inclusive - mask) + poff ; rkeff adds BIG to unselected
            nc.vector.tensor_tensor(out=rank[:], in0=src[:], in1=mask[:], op=Alu.subtract)
            nc.vector.tensor_scalar(
                out=rank[:], in0=rank[:], scalar1=poff[:], scalar2=None, op0=Alu.add
            )
            nc.vector.tensor_scalar(
                out=im[:], in0=mask[:], scalar1=1.0, scalar2=-BIG,
                op0=Alu.subtract, op1=Alu.mult,
            )
            nc.vector.tensor_tensor(out=rkeff[:], in0=rank[:], in1=im[:], op=Alu.add)
            # div = round((rkeff - 63.5)/128)  (== rkeff // 128, exact)
            nc.vector.tensor_scalar(
                out=div[:], in0=rkeff[:], scalar1=63.5, scalar2=1.0 / 128.0,
                op0=Alu.subtract, op1=Alu.mult,
            )
            # rounding const 1.5*2^23 keeps the sum in [2^23, 2^24) even for
            # slightly negative y (quantum below 2^23 is 0.5, not 1)
            nc.vector.tensor_scalar(
                out=div[:], in0=div[:], scalar1=float(3 * 2.0**22), scalar2=None,
                op0=Alu.add,
            )
            nc.vector.tensor_scalar(
                out=div[:], in0=div[:], scalar1=float(3 * 2.0**22), scalar2=None,
                op0=Alu.subtract,
            )
            # rmod = rkeff - 128*div
            nc.vector.tensor_scalar(
                out=rmod[:], in0=div[:], scalar1=128.0, scalar2=None, op0=Alu.mult
            )
            nc.vector.tensor_tensor(out=rmod[:], in0=rkeff[:], in1=rmod[:], op=Alu.subtract)
            # rhs_all[p,t,u] = (div[p,t]==u) * j(p,t)
            nc.vector.tensor_tensor(
                out=dsel[:].rearrange("p (t u) -> p t u", u=U),
                in0=div[:].to_broadcast([128, JT, U]),
                in1=i35[:].unsqueeze(1).broadcast_to([128, JT, U]),
                op=Alu.is_equal,
            )
            nc.vector.tensor_tensor(
                out=rhs_all[:].rearrange("p (t u) -> p t u", u=U),
                in0=dsel[:].rearrange("p (t u) -> p t u", u=U),
                in1=jval[:].to_broadcast([128, JT, U]),
                op=Alu.mult,
            )
            # permutation matmuls: list[q,u] = sum_{p,t} [rmod==q]*rhs[p,t,u]
            list_ps = psp.tile([128, U], dt.float32, tag="ps0", name="list_ps")
            for t in range(JT):
                lt = ltp.tile([128, 128], dt.float32, tag="lt")
                nc.vector.tensor_scalar(
                    out=lt[:], in0=i128[:],
                    scalar1=rmod[:, t : t + 1], scalar2=None, op0=Alu.is_equal,
                )
                if debug and t == 0:
                    nc.sync.dma_start(dbg["lt0"][:], lt[:])
                nc.tensor.matmul(
                    list_ps[:], lhsT=lt[:], rhs=rhs_all[:, t * U : (t + 1) * U],
                    start=(t == 0), stop=(t == JT - 1),
                )
            nc.vector.tensor_copy(out=list_sb[:], in_=list_ps[:])
            # pad slots (slot_rank >= cnt) -> DFF
            nc.vector.tensor_scalar(
                out=ge3[:, :JT], in0=scores[:], scalar1=thr_f[:], scalar2=0.0,
                op0=Alu.is_ge, op1=Alu.add, accum_out=cnt_col[:],
            )
            nc.gpsimd.partition_all_reduce(
                cnt_all[:], cnt_col[:], channels=128, reduce_op=bass_isa.ReduceOp.add
            )
            nc.vector.tensor_scalar(
                out=padi[:], in0=islot[:], scalar1=cnt_all[:], scalar2=float(DFF),
                op0=Alu.is_ge, op1=Alu.mult,
            )
            nc.vector.tensor_tensor(out=list_sb[:], in0=list_sb[:], in1=padi[:], op=Alu.add)
            nc.vector.tensor_copy(out=list_i32[:], in_=list_sb[:])

            if debug:
                for name, src_ in [
                    ("scores", scores), ("mask", mask), ("rank", rank),
                    ("rkeff", rkeff), ("div", div), ("rmod", rmod),
                    ("list", list_sb), ("cnt", cnt_all), ("poff", poff),
                    ("npc", npc), ("thrf", thr_f),
                ]:
                    nc.sync.dma_start(dbg[name][:], src_[:])
                nc.sync.dma_start(dbg["listi"][:], list_i32[:].bitcast(dt.float32).bitcast(dt.int32))
                nc.sync.dma_start(dbg["dsel"][:], dsel[:])
                nc.sync.dma_start(dbg["rhsall"][:], rhs_all[:])
                nc.sync.dma_start(dbg["i128"][:], i128[:])
                nc.sync.dma_start(dbg["jval"][:], jval[:])

            # ---- phase D2: gathers + sparse GEMM ----
            for u in range(U):
                nc.gpsimd.indirect_dma_start(
                    out=wg[:, u * DSH : (u + 1) * DSH],
                    out_offset=None,
                    in_=wt[:, :],
                    in_offset=bass.IndirectOffsetOnAxis(
                        ap=list_i32[:, u : u + 1], axis=0
                    ),
                )
            if debug:
                wgf = pp.tile([128, U * DSH], dt.float32, tag="wgf")
                nc.vector.tensor_copy(out=wgf[:], in_=wg[:])
                nc.sync.dma_start(dbg["wg"][:], wgf[:])
            for g in range(NG):
                psums = [
                    psp.tile([DW, SCH], dt.float32, tag=f"ps{q}", name=f"ps_g{g}_q{q}")
                    for q in range(CL * DT)
                ]
                for u in range(U):
                    xgu = xgp.tile([128, SG], mmdt, tag="xgu")
                    nc.gpsimd.indirect_dma_start(
                        out=xgu[:],
                        out_offset=None,
                        in_=xt[:, :],
                        in_offset=bass.IndirectOffsetOnAxis(
                            ap=list_i32[:, u : u + 1], axis=0
                        ),
                        element_offset=g * SG,
                    )
                    for cl in range(CL):
                        for d in range(DT):
                            nc.tensor.matmul(
                                psums[cl * DT + d][:],
                                lhsT=wg[:, u * DSH + d * DW : u * DSH + (d + 1) * DW],
                                rhs=xgu[:, cl * SCH : (cl + 1) * SCH],
                                start=(u == 0),
                                stop=(u == U - 1),
                            )
                for cl in range(CL):
                    for d in range(DT):
                        ot = otp.tile([DW, SCH], dt.float32)
                        nc.scalar.copy(ot[:], psums[cl * DT + d][:])
                        c = g * CL + cl
                        nc.sync.dma_start(
                            outT[d * DW : (d + 1) * DW, c * SCH : (c + 1) * SCH],
                            ot[:],
                        )

    nc.compile()
    return nc


def _get_program(cfg):
    key = (cfg["name"], cfg.get("mm_dtype", MM_DTYPE))
    if key not in _cache:
        _cache[key] = _build_program(cfg)
    return _cache[key]


def _stage_inputs(x, W, cfg):
    """Host-side sharding/layout. Returns per-core in_maps."""
    DFF = cfg["dff"]
    S = cfg["s"]
    D = cfg["d"]
    DSH = D // N_CORES
    SSH = S // N_CORES

    x2d = np.ascontiguousarray(np.asarray(x, dtype=np.float32).reshape(S, DFF))
    Wf = np.asarray(W, dtype=np.float32)

    xT = np.ascontiguousarray(x2d.T)          # [DFF, S]
    WT = np.ascontiguousarray(Wf.T)           # [DFF, D]

    if cfg.get("mm_dtype", MM_DTYPE) == "f32":
        np_mm = np.float32
    else:
        import ml_dtypes

        np_mm = ml_dtypes.bfloat16

    # append one zero row (gather pad target; W row MUST be zero)
    xT_e = np.zeros((DFF + 1, S), dtype=np_mm)
    xT_e[:DFF] = xT.astype(np_mm)
    WT_e = np.zeros((DFF + 1, D), dtype=np_mm)
    WT_e[:DFF] = WT.astype(np_mm)

    lst = np.tril(np.ones((128, 128), dtype=np.float32), k=-1)
    lstT = np.ascontiguousarray(lst.T)  # lhsT layout: [p, q] with p<q ones

    in_maps = []
    for c in range(N_CORES):
        in_maps.append(
            {
                "xs": np.ascontiguousarray(xT[:, c * SSH : (c + 1) * SSH]),
                "xt": xT_e,
                "wt": np.ascontiguousarray(WT_e[:, c * DSH : (c + 1) * DSH]),
                "lst": lstT,
            }
        )
    return in_maps


def run_cfg(x, W, cfg, trace=False, trace_kwargs=None):
    """Run the kernel for a given cfg; returns (out, BassKernelResults)."""
    from concourse.bass_utils import run_bass_kernel_spmd

    S, D = cfg["s"], cfg["d"]
    DSH = D // N_CORES
    nc = _get_program(cfg)
    in_maps = _stage_inputs(x, W, cfg)
    res = run_bass_kernel_spmd(
        nc,
        in_maps,
        core_ids=list(range(N_CORES)),
        trace=trace,
        **(trace_kwargs or {}),
    )
    outT = np.concatenate([res.results[c]["outT"] for c in range(N_CORES)], axis=0)
    out = np.ascontiguousarray(outT.T).reshape(1, S, D).astype(np.float32)
    return out, res


def kernel(x, W):
    out, _ = run_cfg(x, W, FULL_CFG)
    return out
